# revision 22
# baseline (speedup 1.0000x reference)
"""DDiT block (adaLN attention + MLP) on 8 Trainium2 NeuronCores.

Sharding: cores 0-3 -> batch 0, cores 4-7 -> batch 1. Within a 4-core
batch group: attention is sharded by heads (4 heads/core, full sequence);
after the attention out-projection a grouped ReduceScatter sums the
per-head partial outputs and hands each core a 512-token slice, on which
it runs the (token-sharded) MLP.

Host prep folds the adaLN modulation into weights/biases:
  - ada = c @ ada_w.T + ada_b is computed on host (12 MFLOP)
  - LN scale A = norm_w * (1 + sc) is folded into the columns of
    w_qkv / mlp_w1, so the device LN emits the unit-normalized value
  - the shift's contribution to each linear layer is folded into that
    layer's bias (B @ W.T); gates g_msa / g_mlp into w_out / mlp_w2
  - bias2 (g_mlp*mlp_b2) is folded into the residual x_res

Precision split (error budget measured against the fp32 reference):
  - attention runs in fp8e4m3: q/k/v projections, probs, attn@V and the
    out-projection all use DoubleRow fp8 matmuls (2 contraction k-tiles
    per pass). Scores stay bf16 (PE output-bound, fp8 gains nothing).
    NOTE this stack's fp8e4 is the IEEE variant: max finite 240, above
    that the convert yields inf - all scales chosen to stay under ~100.
  - the MLP stays bf16: its branch magnitude is ~20x the attention
    branch, fp8 there alone costs ~2-2.5e-2 relative error.
  - q/k carry a x32 weight scale (scores x1024, folded into the exp
    scale); the v path's x16 cancels in the softmax division because
    the denominator-ones column is also 16.

Device pipeline per core: token-major LN1 (bn_stats) -> PE-transpose ->
fp8 q,k (feature-major) + v (token-major) projections, interleaved per
token-block so the PE starts before LN1 finishes -> per head: scoresT =
K@Q.T bf16 (2-head packed via tile_position), exp(s-2) to fp8 (ACT
exact; 1/3 Schraudolph+copy on DVE), attn@V fp8 DoubleRow with a
16s-augmented V giving the softmax denominator for free (denominator
reciprocal broadcast via PE into the same PSUM bank), delayed division
-> fp8 out-projection -> ReduceScatter -> residual + LN2 -> bf16 MLP,
split 384/128 tokens so the first pass overlaps the last ReduceScatter
-> residual.
"""

import numpy as np

import concourse.bass as bass
import concourse.mybir as mybir
import concourse.tile as tile
from concourse import bacc
from concourse.bass_utils import run_bass_kernel_spmd
from concourse.masks import make_identity

B, S, D, H, HD = 2, 2048, 1024, 16, 64
DFF = 4 * D
TOK = S // 4          # tokens per core for the MLP phase
EPS = 1e-5
GROUPS = [[0, 1, 2, 3], [4, 5, 6, 7]]
F32 = mybir.dt.float32
BF16 = mybir.dt.bfloat16
FP8 = mybir.dt.float8e4
I16 = mybir.dt.int16
AF = mybir.ActivationFunctionType
ALU = mybir.AluOpType
DR = mybir.MatmulPerfMode.DoubleRow

SQK = 32.0   # scale on w_q/w_k (q,k carry x32 each; folded into exp scale)
SV = 16.0    # scale on w_v (cancels in softmax division)
SO = 64.0    # scale on gated w_out
S1 = 32.0    # scale on mlp_w1 hi/lo fp8 pair (descale folded into gelu)
SHIFT = 2.0  # exp(s - SHIFT); cancels in softmax, keeps probs < fp8 max 240

# bf16 Schraudolph exp(ps/8192 - SHIFT): bits = ps*a + b
EXP_A = 23.083128 / 1024.0
EXP_B = 16250.5 - 184.665 * SHIFT

_CACHE = {}


# ---------------------------------------------------------------- host prep

def _f(v):
    return np.ascontiguousarray(np.asarray(v, dtype=np.float32))


def _bf(a):
    import ml_dtypes
    return np.ascontiguousarray(a.astype(ml_dtypes.bfloat16))


def _q8(a):
    import ml_dtypes
    return np.ascontiguousarray(
        np.clip(a, -240.0, 240.0).astype(ml_dtypes.float8_e4m3fn))


def host_prep(inp):
    x, c = _f(inp["x"]), _f(inp["c"])
    norm1_w, norm2_w = _f(inp["norm1_w"]), _f(inp["norm2_w"])
    w_qkv, w_out = _f(inp["w_qkv"]), _f(inp["w_out"])
    mlp_w1, mlp_b1 = _f(inp["mlp_w1"]), _f(inp["mlp_b1"])
    mlp_w2, mlp_b2 = _f(inp["mlp_w2"]), _f(inp["mlp_b2"])
    ada_w, ada_b = _f(inp["ada_w"]), _f(inp["ada_b"])

    ada = c @ ada_w.T + ada_b                      # [B, 6D]
    sh_msa, sc_msa, g_msa, sh_mlp, sc_mlp, g_mlp = np.split(ada, 6, axis=1)
    A1 = norm1_w[None] * (1.0 + sc_msa)            # [B, D]
    A2 = norm2_w[None] * (1.0 + sc_mlp)
    bias_qkv = sh_msa @ w_qkv.T                    # [B, 3D]
    bias1 = mlp_b1[None] + sh_mlp @ mlp_w1.T       # [B, DFF]
    bias2 = g_mlp * mlp_b2[None]                   # [B, D]

    wq, wk, wv = w_qkv[0:D], w_qkv[D:2 * D], w_qkv[2 * D:3 * D]

    in_maps = []
    for cid in range(8):
        b, r = cid // 4, cid % 4
        hsl = slice(256 * r, 256 * r + 256)
        woutg = g_msa[b][:, None] * w_out          # [D, D]
        w2g = g_mlp[b][:, None] * mlp_w2           # [D, DFF]
        # A2-folded w1 as an fp8 hi/lo pair, rt-major blocks [32, D, 128]
        w1A = (S1 * A2[b][:, None] * mlp_w1.T).reshape(
            D, 32, 128).transpose(1, 0, 2)         # [32, D, 128]
        w1hi = _q8(w1A)
        w1lo = _q8(w1A - np.asarray(w1hi, dtype=np.float32))
        in_maps.append({
            "x_b": _bf(x[b]),
            "x_res": np.ascontiguousarray(np.concatenate(
                [x[b][512 * t2 + 128 * r:512 * t2 + 128 * r + 128]
                 for t2 in range(4)]) + bias2[b][None, :]),
            "wqkT": _q8(SQK * A1[b][:, None]
                        * np.vstack([wq[hsl], wk[hsl]]).T),        # [D, 512]
            "bias_qk": np.ascontiguousarray(SQK * np.concatenate(
                [bias_qkv[b, hsl],
                 bias_qkv[b, D + 256 * r:D + 256 * r + 256]])),    # [512]
            "wvT": _q8(SV * A1[b][:, None] * wv[hsl].T),           # [D, 256]
            "bias_v": np.ascontiguousarray(
                SV * bias_qkv[b, 2 * D + 256 * r:2 * D + 256 * r + 256]),
            "woutT": _q8(SO * woutg[:, hsl].T),                    # [256, D]
            "w1hi": w1hi,                                          # [32, D, 128]
            "w1lo": w1lo,
            "bias1": np.ascontiguousarray(bias1[b]),
            "w2gT": _bf(w2g.T.copy()),                             # [DFF, D]
        })
    return in_maps


# ------------------------------------------------------------- device build

def _bc(ap, p=128):
    """Broadcast a DRAM row AP across p partitions (stride-0 partition dim)."""
    return bass.AP(tensor=ap.tensor, offset=ap.offset,
                   ap=[[0, p]] + [list(d) for d in ap.ap])


def build_program(reps=1):
    nc = bacc.Bacc("TRN2", target_bir_lowering=False, debug=False, num_devices=8)

    x_d = nc.dram_tensor("x_b", [S, D], BF16, kind="ExternalInput")
    xr_d = nc.dram_tensor("x_res", [TOK, D], F32, kind="ExternalInput")
    wqk_d = nc.dram_tensor("wqkT", [D, 512], FP8, kind="ExternalInput")
    bqk_d = nc.dram_tensor("bias_qk", [512], F32, kind="ExternalInput")
    wv_d = nc.dram_tensor("wvT", [D, 256], FP8, kind="ExternalInput")
    bv_d = nc.dram_tensor("bias_v", [256], F32, kind="ExternalInput")
    wo_d = nc.dram_tensor("woutT", [256, D], FP8, kind="ExternalInput")
    w1h_d = nc.dram_tensor("w1hi", [32, D, 128], FP8, kind="ExternalInput")
    w1l_d = nc.dram_tensor("w1lo", [32, D, 128], FP8, kind="ExternalInput")
    b1_d = nc.dram_tensor("bias1", [DFF], F32, kind="ExternalInput")
    w2_d = nc.dram_tensor("w2gT", [DFF, D], BF16, kind="ExternalInput")
    out_d = nc.dram_tensor("out", [TOK, D], F32, kind="ExternalOutput")

    with tile.TileContext(nc, num_cores=8) as tc:
        for _ in range(reps):
            _body(nc, tc, x_d, xr_d, wqk_d, bqk_d, wv_d, bv_d,
                  wo_d, w1h_d, w1l_d, b1_d, w2_d, out_d)
    nc.compile()
    return nc


def _body(nc, tc, x_d, xr_d, wqk_d, bqk_d, wv_d, bv_d,
          wo_d, w1h_d, w1l_d, b1_d, w2_d, out_d):
    mm = nc.tensor.matmul

    from contextlib import ExitStack
    with ExitStack() as outer:
        consts = outer.enter_context(tc.tile_pool(name="consts", bufs=1))
        mlpre = outer.enter_context(tc.tile_pool(name="mlpre", bufs=1))
        x2 = [mlpre.tile([128, D], BF16, tag=f"x2_{t}", name=f"x2_{t}")
              for t in range(4)]
        h2h = mlpre.tile([128, 8, TOK], FP8, tag="h2h", name="h2h")
        h2l = mlpre.tile([128, 8, TOK], FP8, tag="h2l", name="h2l")
        w1pool = outer.enter_context(tc.tile_pool(name="w1pool", bufs=1))
        lnt = outer.enter_context(tc.tile_pool(name="lnt", bufs=2))
        psT2 = outer.enter_context(
            tc.tile_pool(name="psT2", bufs=1, space="PSUM"))
        w1h_sb = [w1pool.tile([128, 8, 128], FP8, tag=f"w1h_{rt}", name=f"w1h_{rt}")
                  for rt in range(32)]
        w1l_sb = [w1pool.tile([128, 8, 128], FP8, tag=f"w1l_{rt}", name=f"w1l_{rt}")
                  for rt in range(32)]
        dram = outer.enter_context(tc.tile_pool(name="dram", bufs=1, space="DRAM"))

        # ---- constants
        ident = consts.tile([128, 128], BF16, tag="ident", name="ident")
        make_identity(nc, ident)
        eps_t = consts.tile([128, 1], F32, tag="eps", name="eps")
        nc.vector.memset(eps_t, EPS)
        nsh_t = consts.tile([128, 1], F32, tag="nsh", name="nsh")
        nc.vector.memset(nsh_t, -SHIFT)
        ones_r = consts.tile([1, 64], BF16, tag="ones_r", name="ones_r")
        nc.vector.memset(ones_r, 1.0)
        bvbc = consts.tile([128, 256], F32, tag="bvbc", name="bvbc")
        bqk_t = consts.tile([128, 4], F32, tag="bqk", name="bqk")
        b1_t = consts.tile([128, 32], F32, tag="b1t", name="b1t")

        # ---- DRAM scratch for the chunked collective (one tile per q-block)
        y_part = [dram.tile([512, D], BF16, tag=f"y_part{i}", name=f"y_part{i}")
                  for i in range(4)]
        y_sum = [dram.tile([128, D], BF16, tag=f"y_sum{i}", name=f"y_sum{i}")
                 for i in range(4)]

        with ExitStack() as attctx:
            wpool = attctx.enter_context(tc.tile_pool(name="wpool", bufs=1))
            acts = attctx.enter_context(tc.tile_pool(name="acts", bufs=1))

            wqk_sb = [wpool.tile([128, 2, 512], FP8, tag=f"wqk{k}", name=f"wqk{k}")
                      for k in range(4)]
            wv_sb = [wpool.tile([128, 2, 256], FP8, tag=f"wv{k}", name=f"wv{k}")
                     for k in range(4)]
            wo_sb = wpool.tile([128, 2, D], FP8, tag="wo", name="wo")

            qkT = [acts.tile([128, S], BF16, tag=f"qkT{rt}", name=f"qkT{rt}")
                   for rt in range(4)]
            v_aug = [acts.tile([128, 2, 4, 72], FP8, tag=f"vaug{m}", name=f"vaug{m}")
                     for m in range(8)]
            attnT = acts.tile([128, 2, S], FP8, tag="attnT", name="attnT")

            # ========= P1: LN1 + transpose, interleaved with P2/P3 ===========
            with tc.tile_pool(name="hTp", bufs=1) as hTp, \
                 tc.tile_pool(name="lnp", bufs=3) as lnp, \
                 tc.tile_pool(name="psT", bufs=2, space="PSUM") as psT, \
                 tc.tile_pool(name="psQK", bufs=3, space="PSUM") as psQK, \
                 tc.tile_pool(name="psV", bufs=2, space="PSUM") as psV:
                hT = hTp.tile([128, 8, S], FP8, tag="hT", name="hT")
                for tt in range(16):
                    xt = lnp.tile([128, D], BF16, tag="xt", name="xt")
                    nc.sync.dma_start(out=xt, in_=x_d[tt * 128:(tt + 1) * 128, :])
                    st = lnp.tile([128, 2, 6], F32, tag="st", name="st")
                    xg = xt.rearrange("p (g d) -> p g d", g=2)
                    for g in range(2):
                        nc.vector.bn_stats(out=st[:, g, :], in_=xg[:, g, :])
                    mv = lnp.tile([128, 2], F32, tag="mv", name="mv")
                    nc.vector.bn_aggr(out=mv, in_=st)
                    rstd = lnp.tile([128, 1], F32, tag="rstd", name="rstd")
                    nc.scalar.activation(out=rstd, in_=mv[:, 1:2],
                                         func=AF.Sqrt, bias=eps_t, scale=1.0)
                    nc.vector.reciprocal(out=rstd, in_=rstd)
                    ht = lnp.tile([128, D], BF16, tag="ht", name="ht")
                    if tt % 2 == 0:
                        nc.vector.tensor_scalar(out=ht, in0=xt,
                                                scalar1=mv[:, 0:1],
                                                scalar2=rstd, op0=ALU.subtract,
                                                op1=ALU.mult)
                    else:
                        negmr = lnp.tile([128, 1], F32, tag="negmr",
                                         name="negmr")
                        nc.vector.tensor_scalar(out=negmr, in0=mv[:, 0:1],
                                                scalar1=rstd, scalar2=-1.0,
                                                op0=ALU.mult, op1=ALU.mult)
                        nc.scalar.activation(out=ht, in_=xt, func=AF.Identity,
                                             scale=rstd, bias=negmr)
                    for dg in range(2):
                        pt4 = psT.tile([128, 4, 128], BF16, tag="pt", name="pt")
                        for i in range(4):
                            dc = 4 * dg + i
                            nc.tensor.transpose(
                                pt4[:, i, :], ht[:, dc * 128:(dc + 1) * 128],
                                ident)
                        dst = hT[:, 4 * dg:4 * dg + 4, tt * 128:(tt + 1) * 128]
                        nc.scalar.copy(out=dst, in_=pt4)

                nc.sync.dma_start(out=bvbc, in_=_bc(bv_d[:]))
                nc.sync.dma_start(out=bqk_t, in_=bass.AP(
                    tensor=bqk_d[:].tensor, offset=0, ap=[[1, 128], [128, 4]]))
                nc.sync.dma_start(out=b1_t, in_=bass.AP(
                    tensor=b1_d[:].tensor, offset=0, ap=[[1, 128], [128, 32]]))
                for kp in range(4):
                    nc.sync.dma_start(
                        out=wqk_sb[kp],
                        in_=wqk_d[kp * 256:(kp + 1) * 256, :].rearrange(
                            "(s p) n -> p s n", s=2))
                    nc.sync.dma_start(
                        out=wv_sb[kp],
                        in_=wv_d[kp * 256:(kp + 1) * 256, :].rearrange(
                            "(s p) n -> p s n", s=2))
                nc.sync.dma_start(
                    out=wo_sb,
                    in_=wo_d[:, :].rearrange("(s p) d -> p s d", s=2))

                # ===== P2/P3: q,k then v projections, per token-block =========
                for tb in range(4):
                    for rt in range(4):
                        pm = psQK.tile([128, 512], F32, tag="pm", name="pm")
                        for kp in range(4):
                            mm(pm, lhsT=wqk_sb[kp][:, :, rt * 128:(rt + 1) * 128],
                               rhs=hT[:, 2 * kp:2 * kp + 2,
                                      tb * 512:(tb + 1) * 512],
                               start=(kp == 0), stop=(kp == 3), perf_mode=DR)
                        nc.scalar.activation(
                            out=qkT[rt][:, tb * 512:(tb + 1) * 512],
                            in_=pm, func=AF.Identity,
                            bias=bqk_t[:, rt:rt + 1], scale=1.0)
                    for tt in range(4 * tb, 4 * tb + 4):
                        pv = psV.tile([128, 256], F32, tag="pmv", name="pmv")
                        for kp in range(4):
                            mm(pv, lhsT=hT[:, 2 * kp:2 * kp + 2,
                                          tt * 128:(tt + 1) * 128],
                               rhs=wv_sb[kp],
                               start=(kp == 0), stop=(kp == 3), perf_mode=DR)
                        if tt % 2 == 0:
                            nc.gpsimd.memset(v_aug[tt // 2], SV)
                        nc.vector.tensor_tensor(
                            out=v_aug[tt // 2][:, tt % 2, :, 0:64],
                            in0=pv.rearrange("p (h d) -> p h d", h=4),
                            in1=bvbc.rearrange("p (h d) -> p h d", h=4),
                            op=ALU.add)

            # ================= P4/P5: attention + out-projection ==============
            ln2_holder = []
            with tc.tile_pool(name="attp", bufs=2) as attp, \
                 tc.tile_pool(name="att2", bufs=2) as att2, \
                 tc.tile_pool(name="psS", bufs=2, space="PSUM") as psS, \
                 tc.tile_pool(name="psN", bufs=2, space="PSUM") as psN, \
                 tc.tile_pool(name="psO", bufs=1, space="PSUM") as psO:
                def ln2_chunk(t2):
                    ys = lnt.tile([128, D], BF16, tag="ys", name="ys")
                    nc.sync.dma_start(out=ys, in_=y_sum[t2][:])
                    xr = lnt.tile([128, D], F32, tag="xr", name="xr")
                    nc.sync.dma_start(
                        out=xr, in_=xr_d[t2 * 128:(t2 + 1) * 128, :])
                    nc.vector.tensor_tensor(out=x2[t2], in0=xr, in1=ys,
                                            op=ALU.add)
                    st2 = lnt.tile([128, 2, 6], F32, tag="st2", name="st2")
                    xg2 = x2[t2].rearrange("p (g d) -> p g d", g=2)
                    for g in range(2):
                        nc.vector.bn_stats(out=st2[:, g, :], in_=xg2[:, g, :])
                    mv2 = lnt.tile([128, 2], F32, tag="mv2", name="mv2")
                    nc.vector.bn_aggr(out=mv2, in_=st2)
                    rstd2 = lnt.tile([128, 1], F32, tag="rstd2", name="rstd2")
                    nc.scalar.activation(out=rstd2, in_=mv2[:, 1:2], func=AF.Sqrt,
                                         bias=eps_t, scale=1.0)
                    nc.vector.reciprocal(out=rstd2, in_=rstd2)
                    h2 = lnt.tile([128, D], BF16, tag="h2", name="h2")
                    nc.vector.tensor_scalar(out=h2, in0=x2[t2],
                                            scalar1=mv2[:, 0:1],
                                            scalar2=rstd2, op0=ALU.subtract,
                                            op1=ALU.mult)
                    for dg in range(2):
                        pt24 = psT2.tile([128, 4, 128], BF16, tag="pt2",
                                         name="pt2")
                        for i in range(4):
                            dc = 4 * dg + i
                            nc.tensor.transpose(
                                pt24[:, i, :], h2[:, dc * 128:(dc + 1) * 128],
                                ident)
                        dsth = h2h[:, 4 * dg:4 * dg + 4,
                                   t2 * 128:(t2 + 1) * 128]
                        nc.scalar.copy(out=dsth, in_=pt24)
                        dstl = h2l[:, 4 * dg:4 * dg + 4,
                                   t2 * 128:(t2 + 1) * 128]
                        nc.vector.tensor_tensor(out=dstl, in0=pt24, in1=dsth,
                                                op=ALU.subtract)

                for qb in range(4):
                    qsl = slice(qb * 512, (qb + 1) * 512)
                    exp_t = {}
                    for pair in range(2):
                        for kt in range(16):
                            for sub in range(2):
                                h = 2 * pair + sub
                                m = kt // 2
                                psl = slice(sub * 64, (sub + 1) * 64)
                                ps = psS.tile([128, 512], F32, tag=f"scr{sub}",
                                              name=f"scr{sub}")
                                mm(ps,
                                   lhsT=qkT[2 + pair][psl, kt * 128:(kt + 1) * 128],
                                   rhs=qkT[pair][psl, qsl],
                                   start=True, stop=True,
                                   tile_position=(sub * 64, 0))
                                if kt % 2 == 0:
                                    exp_t[(h, m)] = attp.tile(
                                        [128, 2, 512], FP8, tag=f"e{h}_{m}",
                                        name=f"e{h}_{m}")
                                dst = exp_t[(h, m)][:, kt % 2, :]
                                if kt == 9 or kt % 4 == 3:
                                    # bf16 Schraudolph on DVE; fp8 convert on
                                    # DVE (kt==9) or Pool (kt%4==3)
                                    ei = attp.tile([128, 512], I16,
                                                   tag=f"i{sub}", name=f"i{sub}")
                                    nc.vector.tensor_scalar(
                                        out=ei, in0=ps, scalar1=EXP_A,
                                        scalar2=EXP_B, op0=ALU.mult, op1=ALU.add)
                                    cpeng = (nc.vector if kt == 9
                                             else nc.gpsimd)
                                    cpeng.tensor_copy(
                                        out=dst, in_=ei[:].bitcast(BF16))
                                else:
                                    nc.scalar.activation(
                                        out=dst, in_=ps, func=AF.Exp,
                                        scale=0.125 / 1024.0, bias=nsh_t)
                    if qb >= 2:
                        ln2_chunk(qb - 2)
                    for h in range(4):
                        pn = psN.tile([128, 512], F32, tag="num", name="num")
                        for m in range(8):
                            mm(pn[0:72, :], lhsT=v_aug[m][:, :, h, :],
                               rhs=exp_t[(h, m)],
                               start=(m == 0), stop=(m == 7), perf_mode=DR)
                        rc = att2.tile([1, 512], BF16, tag="rc", name="rc")
                        with nc.allow_low_precision(reason="bf16 softmax denom"):
                            nc.vector.reciprocal(out=rc, in_=pn[64:65, :])
                        # broadcast 1/den into rows 64..127 of the same bank
                        mm(pn[64:128, :], lhsT=ones_r, rhs=rc,
                           start=True, stop=True, tile_position=(0, 64))
                        rcb = att2.tile([64, 512], BF16, tag="rcb", name="rcb")
                        nc.vector.tensor_copy(out=rcb, in_=pn[64:128, :])
                        if h % 2 == 0:
                            nc.vector.tensor_tensor(
                                out=attnT[0:64, h // 2, qsl],
                                in0=pn[0:64, :], in1=rcb, op=ALU.mult)
                        else:
                            ad = att2.tile([64, 512], FP8, tag="adiv", name="adiv")
                            nc.vector.tensor_tensor(
                                out=ad, in0=pn[0:64, :], in1=rcb, op=ALU.mult)
                            nc.sync.dma_start(
                                out=attnT[64:128, h // 2, qsl], in_=ad)
                    if qb == 3:
                        ln2_chunk(2)
                    # out-projection for this q-block (token-major partial y)
                    for tt in range(4):
                        tok = qb * 512 + tt * 128
                        yb = att2.tile([128, D], BF16, tag="ysb", name="ysb")
                        for n in range(2):
                            po = psO.tile([128, 512], F32, tag="po", name="po")
                            mm(po, lhsT=attnT[:, :, tok:tok + 128],
                               rhs=wo_sb[:, :, n * 512:(n + 1) * 512],
                               start=True, stop=True, perf_mode=DR)
                            if n == 0:
                                nc.vector.tensor_scalar(
                                    out=yb[:, 0:512], in0=po,
                                    scalar1=1.0 / SO, scalar2=None, op0=ALU.mult)
                            else:
                                nc.vector.tensor_scalar(
                                    out=yb[:, 512:1024], in0=po,
                                    scalar1=1.0 / SO, scalar2=None, op0=ALU.mult)
                        nc.sync.dma_start(
                            out=y_part[qb][tt * 128:(tt + 1) * 128, :], in_=yb)
                    nc.gpsimd.collective_compute(
                        "ReduceScatter", ALU.add, replica_groups=GROUPS,
                        ins=[y_part[qb].opt()], outs=[y_sum[qb].opt()])
                    for rt in range(8 * qb, 8 * qb + 8):
                        nc.sync.dma_start(
                            out=w1h_sb[rt],
                            in_=w1h_d[rt].rearrange("(kc p) r -> p kc r", p=128))
                        nc.sync.dma_start(
                            out=w1l_sb[rt],
                            in_=w1l_d[rt].rearrange("(kc p) r -> p kc r", p=128))
                ln2_holder.append(ln2_chunk)

        # ==================== P8/P9: MLP (bf16) ==============================
        with tc.tile_pool(name="mlpp", bufs=1) as mlpp, \
             tc.tile_pool(name="w2pool", bufs=1) as w2pool, \
             tc.tile_pool(name="mtmp", bufs=2) as mtmp:
            w2_sb = [w2pool.tile([128, D], BF16, tag=f"w2_{kc}", name=f"w2_{kc}")
                     for kc in range(32)]
            for kc in range(32):
                nc.sync.dma_start(out=w2_sb[kc],
                                  in_=w2_d[kc * 128:(kc + 1) * 128, :])
            g1T = [mlpp.tile([128, TOK], BF16, tag=f"g1T{rt}", name=f"g1T{rt}")
                   for rt in range(32)]

            with tc.tile_pool(name="psM1", bufs=3, space="PSUM") as psM1, \
                 tc.tile_pool(name="psM2", bufs=2, space="PSUM") as psM2:
                def w1_pass(c0, c1, tag):
                    for rt in range(32):
                        pm1 = psM1.tile([128, c1 - c0], F32, tag="pm1",
                                        name="pm1")
                        for kp in range(4):
                            ks = slice(2 * kp, 2 * kp + 2)
                            mm(pm1, lhsT=w1h_sb[rt][:, ks, :],
                               rhs=h2h[:, ks, c0:c1],
                               start=(kp == 0), stop=False, perf_mode=DR)
                            mm(pm1, lhsT=w1h_sb[rt][:, ks, :],
                               rhs=h2l[:, ks, c0:c1],
                               start=False, stop=False, perf_mode=DR)
                            mm(pm1, lhsT=w1l_sb[rt][:, ks, :],
                               rhs=h2h[:, ks, c0:c1],
                               start=False, stop=(kp == 3), perf_mode=DR)
                        nc.scalar.activation(out=g1T[rt][:, c0:c1],
                                             in_=pm1, func=AF.Gelu_apprx_tanh,
                                             bias=b1_t[:, rt:rt + 1],
                                             scale=1.0 / S1)

                def w2_chunk(t2):
                    ob = mtmp.tile([128, D], F32, tag="ob", name="ob")
                    for n in range(2):
                        nsl = slice(n * 512, (n + 1) * 512)
                        pm2 = psM2.tile([128, 512], F32, tag="pm2", name="pm2")
                        for kc in range(32):
                            mm(pm2, lhsT=g1T[kc][:, t2 * 128:(t2 + 1) * 128],
                               rhs=w2_sb[kc][:, nsl],
                               start=(kc == 0), stop=(kc == 31))
                        nc.vector.tensor_tensor(out=ob[:, nsl], in0=pm2,
                                                in1=x2[t2][:, nsl], op=ALU.add)
                    nc.sync.dma_start(out=out_d[t2 * 128:(t2 + 1) * 128, :],
                                      in_=ob)

                # pass A: tokens 0..383 (chunks 0-2) — overlaps the last RS
                w1_pass(0, 384, "w1a")
                ln2_holder[0](3)
                for t2 in (0, 1, 2):
                    w2_chunk(t2)
                # pass B: tokens 384..511 (chunk 3)
                w1_pass(384, TOK, "w1b")
                w2_chunk(3)


# ----------------------------------------------------------------- kernel()

def _get_nc():
    if "nc" not in _CACHE:
        _CACHE["nc"] = build_program()
    return _CACHE["nc"]


def kernel(**inputs) -> np.ndarray:
    in_maps = host_prep(inputs)
    nc = _get_nc()
    res = run_bass_kernel_spmd(nc, in_maps, list(range(8)))
    out = np.zeros((B, S, D), np.float32)
    for cid in range(8):
        b, r = cid // 4, cid % 4
        o = res.results[cid]["out"]
        for t2 in range(4):
            out[b, 512 * t2 + 128 * r:512 * t2 + 128 * r + 128] = \
                o[128 * t2:128 * t2 + 128]
    return out


# revision 23
# speedup vs baseline: 1.5267x; 1.5267x over previous
"""DDiT block (adaLN attention + MLP) on 8 Trainium2 NeuronCores.

Sharding: cores 0-3 -> batch 0, cores 4-7 -> batch 1. Within a 4-core
batch group: attention is sharded by heads (4 heads/core, full sequence);
after the attention out-projection a grouped ReduceScatter sums the
per-head partial outputs and hands each core a 512-token slice, on which
it runs the (token-sharded) MLP.

Host prep folds the adaLN modulation into weights/biases:
  - ada = c @ ada_w.T + ada_b is computed on host (12 MFLOP)
  - LN scale A = norm_w * (1 + sc) is folded into the columns of
    w_qkv / mlp_w1, so the device LN emits the unit-normalized value
  - the shift's contribution to each linear layer is folded into that
    layer's bias (B @ W.T); gates g_msa / g_mlp into w_out / mlp_w2
  - bias2 (g_mlp*mlp_b2) is folded into the residual x_res

Precision split (error budget measured against the fp32 reference):
  - attention runs in fp8e4m3: q/k/v projections, probs, attn@V and the
    out-projection all use DoubleRow fp8 matmuls (2 contraction k-tiles
    per pass). Scores stay bf16 (PE output-bound, fp8 gains nothing).
    NOTE this stack's fp8e4 is the IEEE variant: max finite 240, above
    that the convert yields inf - all scales chosen to stay under ~100.
  - the MLP stays bf16: its branch magnitude is ~20x the attention
    branch, fp8 there alone costs ~2-2.5e-2 relative error.
  - q/k carry a x32 weight scale (scores x1024, folded into the exp
    scale); the v path's x16 cancels in the softmax division because
    the denominator-ones column is also 16.

Device pipeline per core: token-major LN1 (bn_stats) -> PE-transpose ->
fp8 q,k (feature-major) + v (token-major) projections, interleaved per
token-block so the PE starts before LN1 finishes -> per head: scoresT =
K@Q.T bf16 (2-head packed via tile_position), exp(s-2) to fp8 (ACT
exact; 1/3 Schraudolph+copy on DVE), attn@V fp8 DoubleRow with a
16s-augmented V giving the softmax denominator for free (denominator
reciprocal broadcast via PE into the same PSUM bank), delayed division
-> fp8 out-projection -> ReduceScatter -> residual + LN2 -> bf16 MLP,
split 384/128 tokens so the first pass overlaps the last ReduceScatter
-> residual.
"""

import numpy as np

import concourse.bass as bass
import concourse.mybir as mybir
import concourse.tile as tile
from concourse import bacc
from concourse.bass_utils import run_bass_kernel_spmd
from concourse.masks import make_identity

B, S, D, H, HD = 2, 2048, 1024, 16, 64
DFF = 4 * D
TOK = S // 4          # tokens per core for the MLP phase
EPS = 1e-5
GROUPS = [[0, 1, 2, 3], [4, 5, 6, 7]]
F32 = mybir.dt.float32
BF16 = mybir.dt.bfloat16
FP8 = mybir.dt.float8e4
I16 = mybir.dt.int16
AF = mybir.ActivationFunctionType
ALU = mybir.AluOpType
DR = mybir.MatmulPerfMode.DoubleRow

SQK = 32.0   # scale on w_q/w_k (q,k carry x32 each; folded into exp scale)
SV = 16.0    # scale on w_v (cancels in softmax division)
SO = 64.0    # scale on gated w_out
SHIFT = 2.0  # exp(s - SHIFT); cancels in softmax, keeps probs < fp8 max 240

# bf16 Schraudolph exp(ps/8192 - SHIFT): bits = ps*a + b
EXP_A = 23.083128 / 1024.0
EXP_B = 16250.5 - 184.665 * SHIFT

_CACHE = {}


# ---------------------------------------------------------------- host prep

def _f(v):
    return np.ascontiguousarray(np.asarray(v, dtype=np.float32))


def _bf(a):
    import ml_dtypes
    return np.ascontiguousarray(a.astype(ml_dtypes.bfloat16))


def _q8(a):
    import ml_dtypes
    return np.ascontiguousarray(
        np.clip(a, -240.0, 240.0).astype(ml_dtypes.float8_e4m3fn))


def host_prep(inp):
    x, c = _f(inp["x"]), _f(inp["c"])
    norm1_w, norm2_w = _f(inp["norm1_w"]), _f(inp["norm2_w"])
    w_qkv, w_out = _f(inp["w_qkv"]), _f(inp["w_out"])
    mlp_w1, mlp_b1 = _f(inp["mlp_w1"]), _f(inp["mlp_b1"])
    mlp_w2, mlp_b2 = _f(inp["mlp_w2"]), _f(inp["mlp_b2"])
    ada_w, ada_b = _f(inp["ada_w"]), _f(inp["ada_b"])

    ada = c @ ada_w.T + ada_b                      # [B, 6D]
    sh_msa, sc_msa, g_msa, sh_mlp, sc_mlp, g_mlp = np.split(ada, 6, axis=1)
    A1 = norm1_w[None] * (1.0 + sc_msa)            # [B, D]
    A2 = norm2_w[None] * (1.0 + sc_mlp)
    bias_qkv = sh_msa @ w_qkv.T                    # [B, 3D]
    bias1 = mlp_b1[None] + sh_mlp @ mlp_w1.T       # [B, DFF]
    bias2 = g_mlp * mlp_b2[None]                   # [B, D]

    wq, wk, wv = w_qkv[0:D], w_qkv[D:2 * D], w_qkv[2 * D:3 * D]

    in_maps = []
    for cid in range(8):
        b, r = cid // 4, cid % 4
        hsl = slice(256 * r, 256 * r + 256)
        woutg = g_msa[b][:, None] * w_out          # [D, D]
        w2g = g_mlp[b][:, None] * mlp_w2           # [D, DFF]
        # A2-folded w1, rt-major blocks [32, D, 128]
        w1A = A2[b][:, None] * mlp_w1.T            # [D, DFF]
        w1blk = _bf(w1A.reshape(D, 32, 128).transpose(1, 0, 2))
        in_maps.append({
            "x_b": _bf(x[b]),
            "x_res": np.ascontiguousarray(np.concatenate(
                [x[b][512 * t2 + 128 * r:512 * t2 + 128 * r + 128]
                 for t2 in range(4)]) + bias2[b][None, :]),
            "wqkT": _q8(SQK * A1[b][:, None]
                        * np.vstack([wq[hsl], wk[hsl]]).T),        # [D, 512]
            "bias_qk": np.ascontiguousarray(SQK * np.concatenate(
                [bias_qkv[b, hsl],
                 bias_qkv[b, D + 256 * r:D + 256 * r + 256]])),    # [512]
            "wvT": _q8(SV * A1[b][:, None] * wv[hsl].T),           # [D, 256]
            "bias_v": np.ascontiguousarray(
                SV * bias_qkv[b, 2 * D + 256 * r:2 * D + 256 * r + 256]),
            "woutT": _q8(SO * woutg[:, hsl].T),                    # [256, D]
            "w1blk": w1blk,                                        # [32, D, 128]
            "bias1": np.ascontiguousarray(bias1[b]),
            "w2gT": _bf(w2g.T.copy()),                             # [DFF, D]
        })
    return in_maps


# ------------------------------------------------------------- device build

def _bc(ap, p=128):
    """Broadcast a DRAM row AP across p partitions (stride-0 partition dim)."""
    return bass.AP(tensor=ap.tensor, offset=ap.offset,
                   ap=[[0, p]] + [list(d) for d in ap.ap])


def build_program(reps=1):
    nc = bacc.Bacc("TRN2", target_bir_lowering=False, debug=False, num_devices=8)

    x_d = nc.dram_tensor("x_b", [S, D], BF16, kind="ExternalInput")
    xr_d = nc.dram_tensor("x_res", [TOK, D], F32, kind="ExternalInput")
    wqk_d = nc.dram_tensor("wqkT", [D, 512], FP8, kind="ExternalInput")
    bqk_d = nc.dram_tensor("bias_qk", [512], F32, kind="ExternalInput")
    wv_d = nc.dram_tensor("wvT", [D, 256], FP8, kind="ExternalInput")
    bv_d = nc.dram_tensor("bias_v", [256], F32, kind="ExternalInput")
    wo_d = nc.dram_tensor("woutT", [256, D], FP8, kind="ExternalInput")
    w1_d = nc.dram_tensor("w1blk", [32, D, 128], BF16, kind="ExternalInput")
    b1_d = nc.dram_tensor("bias1", [DFF], F32, kind="ExternalInput")
    w2_d = nc.dram_tensor("w2gT", [DFF, D], BF16, kind="ExternalInput")
    out_d = nc.dram_tensor("out", [TOK, D], F32, kind="ExternalOutput")

    with tile.TileContext(nc, num_cores=8) as tc:
        for _ in range(reps):
            _body(nc, tc, x_d, xr_d, wqk_d, bqk_d, wv_d, bv_d,
                  wo_d, w1_d, b1_d, w2_d, out_d)
    nc.compile()
    return nc


def _body(nc, tc, x_d, xr_d, wqk_d, bqk_d, wv_d, bv_d,
          wo_d, w1_d, b1_d, w2_d, out_d):
    mm = nc.tensor.matmul

    from contextlib import ExitStack
    with ExitStack() as outer:
        consts = outer.enter_context(tc.tile_pool(name="consts", bufs=1))
        mlpre = outer.enter_context(tc.tile_pool(name="mlpre", bufs=1))
        x2 = [mlpre.tile([128, D], BF16, tag=f"x2_{t}", name=f"x2_{t}")
              for t in range(4)]
        h2T = mlpre.tile([128, 8, TOK], BF16, tag="h2T", name="h2T")
        w1pool = outer.enter_context(tc.tile_pool(name="w1pool", bufs=1))
        lnt = outer.enter_context(tc.tile_pool(name="lnt", bufs=2))
        psT2 = outer.enter_context(
            tc.tile_pool(name="psT2", bufs=1, space="PSUM"))
        w1_sb = [w1pool.tile([128, 8, 128], BF16, tag=f"w1_{rt}", name=f"w1_{rt}")
                 for rt in range(32)]
        dram = outer.enter_context(tc.tile_pool(name="dram", bufs=1, space="DRAM"))

        # ---- constants
        ident = consts.tile([128, 128], BF16, tag="ident", name="ident")
        make_identity(nc, ident)
        eps_t = consts.tile([128, 1], F32, tag="eps", name="eps")
        nc.vector.memset(eps_t, EPS)
        nsh_t = consts.tile([128, 1], F32, tag="nsh", name="nsh")
        nc.vector.memset(nsh_t, -SHIFT)
        ones_r = consts.tile([1, 64], BF16, tag="ones_r", name="ones_r")
        nc.vector.memset(ones_r, 1.0)
        bvbc = consts.tile([128, 256], F32, tag="bvbc", name="bvbc")
        bqk_t = consts.tile([128, 4], F32, tag="bqk", name="bqk")
        b1_t = consts.tile([128, 32], F32, tag="b1t", name="b1t")

        # ---- DRAM scratch for the chunked collective (one tile per q-block)
        y_part = [dram.tile([512, D], BF16, tag=f"y_part{i}", name=f"y_part{i}")
                  for i in range(4)]
        y_sum = [dram.tile([128, D], BF16, tag=f"y_sum{i}", name=f"y_sum{i}")
                 for i in range(4)]

        with ExitStack() as attctx:
            wpool = attctx.enter_context(tc.tile_pool(name="wpool", bufs=1))
            acts = attctx.enter_context(tc.tile_pool(name="acts", bufs=1))

            wqk_sb = [wpool.tile([128, 2, 512], FP8, tag=f"wqk{k}", name=f"wqk{k}")
                      for k in range(4)]
            wv_sb = [wpool.tile([128, 2, 256], FP8, tag=f"wv{k}", name=f"wv{k}")
                     for k in range(4)]
            wo_sb = wpool.tile([128, 2, D], FP8, tag="wo", name="wo")

            qkT = [acts.tile([128, S], BF16, tag=f"qkT{rt}", name=f"qkT{rt}")
                   for rt in range(4)]
            v_aug = [acts.tile([128, 2, 4, 72], FP8, tag=f"vaug{m}", name=f"vaug{m}")
                     for m in range(8)]
            attnT = acts.tile([128, 2, S], FP8, tag="attnT", name="attnT")

            # ========= P1: LN1 + transpose, interleaved with P2/P3 ===========
            with tc.tile_pool(name="hTp", bufs=1) as hTp, \
                 tc.tile_pool(name="lnp", bufs=3) as lnp, \
                 tc.tile_pool(name="psT", bufs=2, space="PSUM") as psT, \
                 tc.tile_pool(name="psQK", bufs=3, space="PSUM") as psQK, \
                 tc.tile_pool(name="psV", bufs=2, space="PSUM") as psV:
                hT = hTp.tile([128, 8, S], FP8, tag="hT", name="hT")
                for tt in range(16):
                    xt = lnp.tile([128, D], BF16, tag="xt", name="xt")
                    nc.sync.dma_start(out=xt, in_=x_d[tt * 128:(tt + 1) * 128, :])
                    st = lnp.tile([128, 2, 6], F32, tag="st", name="st")
                    xg = xt.rearrange("p (g d) -> p g d", g=2)
                    for g in range(2):
                        nc.vector.bn_stats(out=st[:, g, :], in_=xg[:, g, :])
                    mv = lnp.tile([128, 2], F32, tag="mv", name="mv")
                    nc.vector.bn_aggr(out=mv, in_=st)
                    rstd = lnp.tile([128, 1], F32, tag="rstd", name="rstd")
                    nc.scalar.activation(out=rstd, in_=mv[:, 1:2],
                                         func=AF.Sqrt, bias=eps_t, scale=1.0)
                    nc.vector.reciprocal(out=rstd, in_=rstd)
                    ht = lnp.tile([128, D], BF16, tag="ht", name="ht")
                    if tt % 2 == 0:
                        nc.vector.tensor_scalar(out=ht, in0=xt,
                                                scalar1=mv[:, 0:1],
                                                scalar2=rstd, op0=ALU.subtract,
                                                op1=ALU.mult)
                    else:
                        negmr = lnp.tile([128, 1], F32, tag="negmr",
                                         name="negmr")
                        nc.vector.tensor_scalar(out=negmr, in0=mv[:, 0:1],
                                                scalar1=rstd, scalar2=-1.0,
                                                op0=ALU.mult, op1=ALU.mult)
                        nc.scalar.activation(out=ht, in_=xt, func=AF.Identity,
                                             scale=rstd, bias=negmr)
                    for dg in range(2):
                        pt4 = psT.tile([128, 4, 128], BF16, tag="pt", name="pt")
                        for i in range(4):
                            dc = 4 * dg + i
                            nc.tensor.transpose(
                                pt4[:, i, :], ht[:, dc * 128:(dc + 1) * 128],
                                ident)
                        dst = hT[:, 4 * dg:4 * dg + 4, tt * 128:(tt + 1) * 128]
                        nc.scalar.copy(out=dst, in_=pt4)

                nc.sync.dma_start(out=bvbc, in_=_bc(bv_d[:]))
                nc.sync.dma_start(out=bqk_t, in_=bass.AP(
                    tensor=bqk_d[:].tensor, offset=0, ap=[[1, 128], [128, 4]]))
                nc.sync.dma_start(out=b1_t, in_=bass.AP(
                    tensor=b1_d[:].tensor, offset=0, ap=[[1, 128], [128, 32]]))
                for kp in range(4):
                    nc.sync.dma_start(
                        out=wqk_sb[kp],
                        in_=wqk_d[kp * 256:(kp + 1) * 256, :].rearrange(
                            "(s p) n -> p s n", s=2))
                    nc.sync.dma_start(
                        out=wv_sb[kp],
                        in_=wv_d[kp * 256:(kp + 1) * 256, :].rearrange(
                            "(s p) n -> p s n", s=2))
                nc.sync.dma_start(
                    out=wo_sb,
                    in_=wo_d[:, :].rearrange("(s p) d -> p s d", s=2))

                # ===== P2/P3: q,k then v projections, per token-block =========
                for tb in range(4):
                    for rt in range(4):
                        pm = psQK.tile([128, 512], F32, tag="pm", name="pm")
                        for kp in range(4):
                            mm(pm, lhsT=wqk_sb[kp][:, :, rt * 128:(rt + 1) * 128],
                               rhs=hT[:, 2 * kp:2 * kp + 2,
                                      tb * 512:(tb + 1) * 512],
                               start=(kp == 0), stop=(kp == 3), perf_mode=DR)
                        nc.scalar.activation(
                            out=qkT[rt][:, tb * 512:(tb + 1) * 512],
                            in_=pm, func=AF.Identity,
                            bias=bqk_t[:, rt:rt + 1], scale=1.0)
                    for tt in range(4 * tb, 4 * tb + 4):
                        pv = psV.tile([128, 256], F32, tag="pmv", name="pmv")
                        for kp in range(4):
                            mm(pv, lhsT=hT[:, 2 * kp:2 * kp + 2,
                                          tt * 128:(tt + 1) * 128],
                               rhs=wv_sb[kp],
                               start=(kp == 0), stop=(kp == 3), perf_mode=DR)
                        if tt % 2 == 0:
                            nc.gpsimd.memset(v_aug[tt // 2], SV)
                        nc.vector.tensor_tensor(
                            out=v_aug[tt // 2][:, tt % 2, :, 0:64],
                            in0=pv.rearrange("p (h d) -> p h d", h=4),
                            in1=bvbc.rearrange("p (h d) -> p h d", h=4),
                            op=ALU.add)

            # ================= P4/P5: attention + out-projection ==============
            ln2_holder = []
            with tc.tile_pool(name="attp", bufs=2) as attp, \
                 tc.tile_pool(name="att2", bufs=2) as att2, \
                 tc.tile_pool(name="psS", bufs=2, space="PSUM") as psS, \
                 tc.tile_pool(name="psN", bufs=2, space="PSUM") as psN, \
                 tc.tile_pool(name="psO", bufs=1, space="PSUM") as psO:
                def ln2_chunk(t2):
                    ys = lnt.tile([128, D], BF16, tag="ys", name="ys")
                    nc.sync.dma_start(out=ys, in_=y_sum[t2][:])
                    xr = lnt.tile([128, D], F32, tag="xr", name="xr")
                    nc.sync.dma_start(
                        out=xr, in_=xr_d[t2 * 128:(t2 + 1) * 128, :])
                    nc.vector.tensor_tensor(out=x2[t2], in0=xr, in1=ys,
                                            op=ALU.add)
                    st2 = lnt.tile([128, 2, 6], F32, tag="st2", name="st2")
                    xg2 = x2[t2].rearrange("p (g d) -> p g d", g=2)
                    for g in range(2):
                        nc.vector.bn_stats(out=st2[:, g, :], in_=xg2[:, g, :])
                    mv2 = lnt.tile([128, 2], F32, tag="mv2", name="mv2")
                    nc.vector.bn_aggr(out=mv2, in_=st2)
                    rstd2 = lnt.tile([128, 1], F32, tag="rstd2", name="rstd2")
                    nc.scalar.activation(out=rstd2, in_=mv2[:, 1:2], func=AF.Sqrt,
                                         bias=eps_t, scale=1.0)
                    nc.vector.reciprocal(out=rstd2, in_=rstd2)
                    h2 = lnt.tile([128, D], BF16, tag="h2", name="h2")
                    nc.vector.tensor_scalar(out=h2, in0=x2[t2],
                                            scalar1=mv2[:, 0:1],
                                            scalar2=rstd2, op0=ALU.subtract,
                                            op1=ALU.mult)
                    for dg in range(2):
                        pt24 = psT2.tile([128, 4, 128], BF16, tag="pt2",
                                         name="pt2")
                        for i in range(4):
                            dc = 4 * dg + i
                            nc.tensor.transpose(
                                pt24[:, i, :], h2[:, dc * 128:(dc + 1) * 128],
                                ident)
                        dst = h2T[:, 4 * dg:4 * dg + 4,
                                  t2 * 128:(t2 + 1) * 128]
                        if dg == 0:
                            nc.scalar.copy(out=dst, in_=pt24)
                        else:
                            nc.vector.tensor_copy(out=dst, in_=pt24)

                for qb in range(4):
                    qsl = slice(qb * 512, (qb + 1) * 512)
                    exp_t = {}
                    for pair in range(2):
                        for kt in range(16):
                            for sub in range(2):
                                h = 2 * pair + sub
                                m = kt // 2
                                psl = slice(sub * 64, (sub + 1) * 64)
                                ps = psS.tile([128, 512], F32, tag=f"scr{sub}",
                                              name=f"scr{sub}")
                                mm(ps,
                                   lhsT=qkT[2 + pair][psl, kt * 128:(kt + 1) * 128],
                                   rhs=qkT[pair][psl, qsl],
                                   start=True, stop=True,
                                   tile_position=(sub * 64, 0))
                                if kt % 2 == 0:
                                    exp_t[(h, m)] = attp.tile(
                                        [128, 2, 512], FP8, tag=f"e{h}_{m}",
                                        name=f"e{h}_{m}")
                                dst = exp_t[(h, m)][:, kt % 2, :]
                                if kt == 9 or kt % 4 == 3:
                                    # bf16 Schraudolph on DVE; fp8 convert on
                                    # DVE (kt==9) or Pool (kt%4==3)
                                    ei = attp.tile([128, 512], I16,
                                                   tag=f"i{sub}", name=f"i{sub}")
                                    nc.vector.tensor_scalar(
                                        out=ei, in0=ps, scalar1=EXP_A,
                                        scalar2=EXP_B, op0=ALU.mult, op1=ALU.add)
                                    cpeng = (nc.vector if kt == 9
                                             else nc.gpsimd)
                                    cpeng.tensor_copy(
                                        out=dst, in_=ei[:].bitcast(BF16))
                                else:
                                    nc.scalar.activation(
                                        out=dst, in_=ps, func=AF.Exp,
                                        scale=0.125 / 1024.0, bias=nsh_t)
                    if qb >= 2:
                        ln2_chunk(qb - 2)
                    for h in range(4):
                        pn = psN.tile([128, 512], F32, tag="num", name="num")
                        for m in range(8):
                            mm(pn[0:72, :], lhsT=v_aug[m][:, :, h, :],
                               rhs=exp_t[(h, m)],
                               start=(m == 0), stop=(m == 7), perf_mode=DR)
                        rc = att2.tile([1, 512], BF16, tag="rc", name="rc")
                        with nc.allow_low_precision(reason="bf16 softmax denom"):
                            nc.vector.reciprocal(out=rc, in_=pn[64:65, :])
                        # broadcast 1/den into rows 64..127 of the same bank
                        mm(pn[64:128, :], lhsT=ones_r, rhs=rc,
                           start=True, stop=True, tile_position=(0, 64))
                        rcb = att2.tile([64, 512], BF16, tag="rcb", name="rcb")
                        nc.vector.tensor_copy(out=rcb, in_=pn[64:128, :])
                        if h % 2 == 0:
                            nc.vector.tensor_tensor(
                                out=attnT[0:64, h // 2, qsl],
                                in0=pn[0:64, :], in1=rcb, op=ALU.mult)
                        else:
                            ad = att2.tile([64, 512], FP8, tag="adiv", name="adiv")
                            nc.vector.tensor_tensor(
                                out=ad, in0=pn[0:64, :], in1=rcb, op=ALU.mult)
                            nc.sync.dma_start(
                                out=attnT[64:128, h // 2, qsl], in_=ad)
                    if qb == 3:
                        ln2_chunk(2)
                    # out-projection for this q-block (token-major partial y)
                    for tt in range(4):
                        tok = qb * 512 + tt * 128
                        yb = att2.tile([128, D], BF16, tag="ysb", name="ysb")
                        for n in range(2):
                            po = psO.tile([128, 512], F32, tag="po", name="po")
                            mm(po, lhsT=attnT[:, :, tok:tok + 128],
                               rhs=wo_sb[:, :, n * 512:(n + 1) * 512],
                               start=True, stop=True, perf_mode=DR)
                            if n == 0:
                                nc.vector.tensor_scalar(
                                    out=yb[:, 0:512], in0=po,
                                    scalar1=1.0 / SO, scalar2=None, op0=ALU.mult)
                            else:
                                nc.vector.tensor_scalar(
                                    out=yb[:, 512:1024], in0=po,
                                    scalar1=1.0 / SO, scalar2=None, op0=ALU.mult)
                        nc.sync.dma_start(
                            out=y_part[qb][tt * 128:(tt + 1) * 128, :], in_=yb)
                    nc.gpsimd.collective_compute(
                        "ReduceScatter", ALU.add, replica_groups=GROUPS,
                        ins=[y_part[qb].opt()], outs=[y_sum[qb].opt()])
                    for rt in range(8 * qb, 8 * qb + 8):
                        nc.sync.dma_start(
                            out=w1_sb[rt],
                            in_=w1_d[rt].rearrange("(kc p) r -> p kc r", p=128))
                ln2_holder.append(ln2_chunk)

        # ==================== P8/P9: MLP (bf16) ==============================
        with tc.tile_pool(name="mlpp", bufs=1) as mlpp, \
             tc.tile_pool(name="w2pool", bufs=1) as w2pool, \
             tc.tile_pool(name="mtmp", bufs=2) as mtmp:
            w2_sb = [w2pool.tile([128, D], BF16, tag=f"w2_{kc}", name=f"w2_{kc}")
                     for kc in range(32)]
            for kc in range(32):
                nc.sync.dma_start(out=w2_sb[kc],
                                  in_=w2_d[kc * 128:(kc + 1) * 128, :])
            g1T = [mlpp.tile([128, TOK], BF16, tag=f"g1T{rt}", name=f"g1T{rt}")
                   for rt in range(32)]

            with tc.tile_pool(name="psM1", bufs=3, space="PSUM") as psM1, \
                 tc.tile_pool(name="psM2", bufs=2, space="PSUM") as psM2:
                def w1_pass(c0, c1, tag):
                    for rt in range(32):
                        pm1 = psM1.tile([128, c1 - c0], F32, tag="pm1",
                                        name="pm1")
                        for kc in range(8):
                            mm(pm1, lhsT=w1_sb[rt][:, kc, :],
                               rhs=h2T[:, kc, c0:c1],
                               start=(kc == 0), stop=(kc == 7))
                        nc.scalar.activation(out=g1T[rt][:, c0:c1],
                                             in_=pm1, func=AF.Gelu_apprx_tanh,
                                             bias=b1_t[:, rt:rt + 1], scale=1.0)

                def w2_chunk(t2):
                    ob = mtmp.tile([128, D], F32, tag="ob", name="ob")
                    for n in range(2):
                        nsl = slice(n * 512, (n + 1) * 512)
                        pm2 = psM2.tile([128, 512], F32, tag="pm2", name="pm2")
                        for kc in range(32):
                            mm(pm2, lhsT=g1T[kc][:, t2 * 128:(t2 + 1) * 128],
                               rhs=w2_sb[kc][:, nsl],
                               start=(kc == 0), stop=(kc == 31))
                        nc.vector.tensor_tensor(out=ob[:, nsl], in0=pm2,
                                                in1=x2[t2][:, nsl], op=ALU.add)
                    nc.sync.dma_start(out=out_d[t2 * 128:(t2 + 1) * 128, :],
                                      in_=ob)

                # pass A: tokens 0..383 (chunks 0-2) — overlaps the last RS
                w1_pass(0, 384, "w1a")
                ln2_holder[0](3)
                for t2 in (0, 1, 2):
                    w2_chunk(t2)
                # pass B: tokens 384..511 (chunk 3)
                w1_pass(384, TOK, "w1b")
                w2_chunk(3)


# ----------------------------------------------------------------- kernel()

def _get_nc():
    if "nc" not in _CACHE:
        _CACHE["nc"] = build_program()
    return _CACHE["nc"]


def kernel(**inputs) -> np.ndarray:
    in_maps = host_prep(inputs)
    nc = _get_nc()
    res = run_bass_kernel_spmd(nc, in_maps, list(range(8)))
    out = np.zeros((B, S, D), np.float32)
    for cid in range(8):
        b, r = cid // 4, cid % 4
        o = res.results[cid]["out"]
        for t2 in range(4):
            out[b, 512 * t2 + 128 * r:512 * t2 + 128 * r + 128] = \
                o[128 * t2:128 * t2 + 128]
    return out
